# revision 17
# baseline (speedup 1.0000x reference)
"""Trainium2 Bass kernel for the BINN convnet problem (transposed bf16 design).

Computation (per row b of inp, all column indices mod D=128):
    x[b, j]  = (c1[j] * a[b, j+1] - c2[j] * a[b, j-2]) * a[b, j-1]
    out      = x + a @ W_lin.T + b_lin
with c1[j] = w[j,0]*outer[j], c2[j] = w[j,1]*outer[j], outer[j] = w[j,2]
except j==1 where outer is w[1,0].

Defining G[j, (j+1)%D] = c1[j], G[j, (j-2)%D] = -c2[j]:
    g = a @ G.T;  x[b, j] = g[b, j] * a[b, j-1];  out = x + a @ W_lin.T + b_lin

Strategy: pure data parallel across 8 NeuronCores (batch split), computing in
FEATURE-TRANSPOSED space with a row-rolled coordinate change so every on-chip
elementwise op is perfectly aligned:

  - The host uploads aT = shard.T as bf16 [128, nrows]: features live on
    partitions, batch on the free dim.  No on-chip transpose is needed, and
    the j-1 stencil shift becomes a partition shift, which a row-roll of the
    CONSTANT matrices absorbs:
        Gs = roll(G, -1, rows); Ws = roll(W_lin, -1, rows); bs = roll(b_lin, -1)
        gs  = Gs @ aT           (gs[j]  = g^T[j+1])
        ms  = Ws @ aT + bs      (ms[j]  = (a@W^T + b)^T[j+1])
        xs  = aT * gs           (aligned elementwise, no wrap)
        outs = xs + ms          (outs[j] = out^T[j+1])
    The host un-rolls the output: out = roll(outs, +1, rows).T.
  - Per 1024-col subtile: 4 bf16 matmuls (N=512 each, stationary Gs^T / Ws^T
    consts) write gs/ms to PSUM fp32; the evacuation to SBUF bf16 runs at 1x
    (fp32 PSUM source), so it is split between ScalarE and DVE; the bias add
    rides the ms evacuation (ScalarE per-partition activation bias).
  - The mul runs on DVE as bf16 tensor_tensor in 2x_1P mode (16-bit, step 1,
    4B-aligned) over 2048-col groups; the add is split DVE / GpSimd (GpSimd
    is SBUF-only but otherwise idle).
  - bf16 halves the DMA traffic (16 MiB in + 16 MiB out per core).  Consts
    load first on the SP HWDGE ring (so they are not starved behind data
    tiles), then 4 warm-up 2048-col tiles let compute start early, then
    8192-col tiles; stores ride the ACT ring in 4096-col tiles.
"""

import os
import sys

import numpy as np

if os.path.isdir("/opt/trn_rl_repo") and "/opt/trn_rl_repo" not in sys.path:
    sys.path.insert(0, "/opt/trn_rl_repo")

import ml_dtypes

import concourse.mybir as mybir
import concourse.tile as tile
from concourse import bacc
from concourse.bass_utils import run_bass_kernel_spmd

D = 128            # feature dim
N_CORES = 8
SUB = 1024         # cols per PSUM subtile (2 banks per tensor)
GRP = 2048         # cols per DVE group (2 subtiles)
T_WARM = 2048      # cols per warm-up load tile (x4)
N_WARM = 4
T_LOAD = 8192      # cols per steady-state load DMA tile
T_STORE = 4096     # cols per store DMA tile
K_SAFE = 8         # warm-up subtiles on the race-free (non-fused) path
BF16 = mybir.dt.bfloat16
F32 = mybir.dt.float32
NP_BF16 = ml_dtypes.bfloat16


def build_program(nrows: int):
    """Build the single-core Bass program (SPMD across cores)."""
    assert (nrows - N_WARM * T_WARM) % T_LOAD == 0 and nrows % GRP == 0
    ngrp = nrows // GRP

    nc = bacc.Bacc("TRN2", debug=False, target_bir_lowering=False)

    at = nc.declare_dram_parameter("at", [D, nrows], BF16, isOutput=False)
    gst = nc.declare_dram_parameter("gst", [D, D], BF16, isOutput=False)
    wst = nc.declare_dram_parameter("wst", [D, D], BF16, isOutput=False)
    bs = nc.declare_dram_parameter("bs", [D, 1], F32, isOutput=False)
    out = nc.declare_dram_parameter("out", [D, nrows], BF16, isOutput=True)

    with tile.TileContext(nc) as tc:
        with (
            tc.tile_pool(name="const", bufs=1) as const_pool,
            tc.tile_pool(name="aw_sb", bufs=N_WARM) as aw_pool,
            tc.tile_pool(name="a_sb", bufs=3) as a_pool,
            tc.tile_pool(name="o_sb", bufs=3) as o_pool,
            tc.tile_pool(name="xs_sb", bufs=3) as xs_pool,
            tc.tile_pool(name="gs_ps", bufs=2, space="PSUM") as gsps_pool,
            tc.tile_pool(name="ms_ps", bufs=2, space="PSUM") as msps_pool,
        ):
            # --- constants (tiny), FIRST on the SP ring: they complete
            # before the first data tile (FIFO) instead of trickling at
            # packet granularity behind 2 MiB loads ---
            gst_sb = const_pool.tile([D, D], BF16)
            wst_sb = const_pool.tile([D, D], BF16)
            wk_sb = const_pool.tile([D, D], BF16)
            bs_sb = const_pool.tile([D, 1], F32)
            nc.sync.dma_start(out=gst_sb[:], in_=gst[:, :])
            nc.sync.dma_start(out=wst_sb[:], in_=wst[:, :])
            nc.sync.dma_start(out=wk_sb[:], in_=wst[:, :])
            nc.sync.dma_start(out=bs_sb[:], in_=bs[:, :])

            # Per-subtile software pipeline, staged so every engine's queue
            # stays hazard-free:
            #   A(k): PE   gs_ps(k) = Gs @ aT(k)          (2 matmuls)
            #   B(k): DVE  ms_ps(k) = aT(k) * gs_ps(k)    (xs, fp32 in PSUM)
            #   C(k): PE   ms_ps(k) += Ws @ aT(k)         (2 matmuls,
            #              start=False accumulate on top of xs)
            #   D(k): Sc   o(k) = bf16(ms_ps(k) + bs)     (activation bias)
            #   E:    store per T_STORE cols
            # PE's queue per step is [A(k), C(k-1)] so the PE never waits
            # long on B; PSUM: gs 2 bufs + ms 2 bufs = 8 banks exactly.
            nsub = nrows // SUB
            st = {}  # k -> (a_sb, o_sb, ga, go)
            a_sb, o_sb = None, None

            def emit_A_B(k):
                nonlocal a_sb, o_sb
                c0 = k * SUB
                # --- load (SP HWDGE ring): warm-up tiles then 8 KiB tiles ---
                if c0 < N_WARM * T_WARM:
                    if c0 % T_WARM == 0:
                        a_sb = aw_pool.tile([D, T_WARM], BF16, tag="aw")
                        nc.sync.dma_start(
                            out=a_sb[:], in_=at[:, c0 : c0 + T_WARM]
                        )
                    ga = c0 % T_WARM
                else:
                    if (c0 - N_WARM * T_WARM) % T_LOAD == 0:
                        a_sb = a_pool.tile([D, T_LOAD], BF16, tag="a")
                        nc.sync.dma_start(
                            out=a_sb[:], in_=at[:, c0 : c0 + T_LOAD]
                        )
                    ga = (c0 - N_WARM * T_WARM) % T_LOAD
                if c0 % T_STORE == 0:
                    o_sb = o_pool.tile([D, T_STORE], BF16, tag="o")
                go = c0 % T_STORE
                st[k] = (a_sb, o_sb, ga, go)

                gs_ps = gsps_pool.tile([D, SUB], F32, tag="gsps")
                ms_ps = msps_pool.tile([D, SUB], F32, tag="msps")
                for h in range(2):
                    hs = h * 512
                    nc.tensor.matmul(
                        out=gs_ps[:, hs : hs + 512],
                        lhsT=gst_sb[:],
                        rhs=a_sb[:, ga + hs : ga + hs + 512],
                        start=True,
                        stop=True,
                    )
                if k < K_SAFE:
                    # warm-up subtiles: race-free path -- xs to SBUF, the
                    # combine happens in D via a DVE STT (all deps are
                    # ordinary RAW; no PE accumulate onto DVE data)
                    xs_sb = xs_pool.tile([D, SUB], BF16, tag="xs")
                    nc.vector.tensor_mul(
                        out=xs_sb[:], in0=a_sb[:, ga : ga + SUB], in1=gs_ps[:]
                    )
                else:
                    xs_sb = None
                    # xs straight into the ms PSUM tile (fp32, 1x)
                    nc.vector.tensor_mul(
                        out=ms_ps[:], in0=a_sb[:, ga : ga + SUB], in1=gs_ps[:]
                    )
                st[k] += (gs_ps, ms_ps, xs_sb)

            def emit_C(k):
                a_sb, _, ga, _, _, ms_ps, xs_sb = st[k]
                safe = xs_sb is not None
                if not safe:
                    # The scheduler reorders engine queues freely subject to
                    # data deps, so the cross-engine WAW (DVE xs write in
                    # B(k) -> PE accumulate below) needs a REAL dataflow
                    # chain: rewrite column 0 of the ms matmuls' stationary
                    # tile as (xs[:,0:1] * 0.0) + wst[:,0:1] -- same values,
                    # but it READS the mul's output, so mul(k) -> stt(k) ->
                    # msmm(k) is enforced by ordinary RAW tracking.
                    nc.vector.scalar_tensor_tensor(
                        out=wk_sb[:, 0:1],
                        in0=ms_ps[:, 0:1],
                        scalar=0.0,
                        in1=wst_sb[:, 0:1],
                        op0=mybir.AluOpType.mult,
                        op1=mybir.AluOpType.add,
                    )
                for h in range(2):
                    hs = h * 512
                    nc.tensor.matmul(
                        out=ms_ps[:, hs : hs + 512],
                        lhsT=wst_sb[:] if safe else wk_sb[:],
                        rhs=a_sb[:, ga + hs : ga + hs + 512],
                        start=safe,
                        stop=True,
                        skip_group_check=not safe,
                    )

            def emit_D_E(k):
                _, o_sb, _, go, _, ms_ps, xs_sb = st[k]
                if xs_sb is not None:
                    nc.vector.scalar_tensor_tensor(
                        out=o_sb[:, go : go + SUB],
                        in0=xs_sb[:],
                        scalar=bs_sb[:],
                        in1=ms_ps[:],
                        op0=mybir.AluOpType.add,
                        op1=mybir.AluOpType.add,
                    )
                else:
                    nc.scalar.add(
                        out=o_sb[:, go : go + SUB], in_=ms_ps[:], add=bs_sb[:]
                    )
                c1 = (k + 1) * SUB
                if c1 % T_STORE == 0:
                    nc.scalar.dma_start(
                        out=out[:, c1 - T_STORE : c1], in_=o_sb[:]
                    )
                del st[k]

            for step in range(nsub + 2):
                if step < nsub:
                    emit_A_B(step)
                if 1 <= step <= nsub:
                    emit_C(step - 1)
                if step >= 2:
                    emit_D_E(step - 2)

    nc.compile()
    return nc


def make_consts(w: np.ndarray, W_lin: np.ndarray, b_lin: np.ndarray):
    """Host-side constant preparation (all tiny)."""
    w = np.asarray(w, np.float64)
    c1 = w[:, 0] * w[:, 2]
    c2 = w[:, 1] * w[:, 2]
    # column 1 uses w[1,0] as the outer factor (faithful to source)
    c1[1] = w[1, 0] * w[1, 0]
    c2[1] = w[1, 1] * w[1, 0]

    j = np.arange(D)
    G = np.zeros((D, D), np.float64)
    G[j, (j + 1) % D] += c1
    G[j, (j - 2) % D] -= c2

    Gs = np.roll(G, -1, axis=0)
    Ws = np.roll(np.asarray(W_lin, np.float64), -1, axis=0)
    bsv = np.roll(np.asarray(b_lin, np.float32), -1)

    gst = np.ascontiguousarray(Gs.T).astype(NP_BF16)
    wst = np.ascontiguousarray(Ws.T).astype(NP_BF16)
    bs = np.ascontiguousarray(bsv[:, None].astype(np.float32))
    return {"gst": gst, "wst": wst, "bs": bs}


_PROGRAM_CACHE: dict[int, object] = {}
TRACE = False      # test-only: capture NTFF profile on the next kernel() call
TRACE_DIR = None   # test-only: where to keep NTFF/perfetto artifacts
LAST_RESULT = None  # test-only: BassKernelResults of the last run


def _get_program(nrows: int):
    if nrows not in _PROGRAM_CACHE:
        _PROGRAM_CACHE[nrows] = build_program(nrows)
    return _PROGRAM_CACHE[nrows]


def kernel(**inputs) -> np.ndarray:
    inp = np.asarray(inputs["inp"])
    w = np.asarray(inputs["w"], np.float32)
    W_lin = np.asarray(inputs["W_lin"], np.float32)
    b_lin = np.asarray(inputs["b_lin"], np.float32)

    B = inp.shape[0]
    assert inp.shape[1] == D and B % N_CORES == 0
    nrows = B // N_CORES

    consts = make_consts(w, W_lin, b_lin)
    inp_bf = inp.astype(NP_BF16)

    nc = _get_program(nrows)
    in_maps = []
    for i in range(N_CORES):
        at = np.ascontiguousarray(inp_bf[i * nrows : (i + 1) * nrows, :].T)
        in_maps.append({"at": at, **consts})
    res = run_bass_kernel_spmd(
        nc, in_maps, list(range(N_CORES)), trace=TRACE, tmpdir=TRACE_DIR
    )
    global LAST_RESULT
    LAST_RESULT = res
    outs = [
        np.roll(np.asarray(res.results[i]["out"]), 1, axis=0).T.astype(np.float32)
        for i in range(N_CORES)
    ]
    return np.ascontiguousarray(np.concatenate(outs, axis=0))


if __name__ == "__main__":
    # quick smoke test on random data vs numpy
    rng = np.random.default_rng(0)
    B = N_CORES * (N_WARM * T_WARM + T_LOAD)
    inp = rng.standard_normal((B, D)).astype(np.float32)
    w = rng.random((D, 3)).astype(np.float32)
    W_lin = (rng.standard_normal((D, D)) / np.sqrt(D)).astype(np.float32)
    b_lin = (rng.standard_normal(D) * 0.01).astype(np.float32)
    dt = np.ones(1, np.float32)

    actual = kernel(inp=inp, dt=dt, w=w, W_lin=W_lin, b_lin=b_lin)

    a = inp.astype(np.float64)
    c1 = (w[:, 0] * w[:, 2]).astype(np.float64)
    c2 = (w[:, 1] * w[:, 2]).astype(np.float64)
    c1[1] = w[1, 0] * w[1, 0]
    c2[1] = w[1, 1] * w[1, 0]
    ap1 = np.roll(a, -1, 1)
    am2 = np.roll(a, 2, 1)
    am1 = np.roll(a, 1, 1)
    x = (c1 * ap1 - c2 * am2) * am1
    expected = x + a @ W_lin.astype(np.float64).T + b_lin
    err = np.abs(actual - expected).max() / np.abs(expected).max()
    print("scale-relative absmax err:", err)


# revision 18
# speedup vs baseline: 1.0222x; 1.0222x over previous
"""Trainium2 Bass kernel for the BINN convnet problem (transposed bf16 design).

Computation (per row b of inp, all column indices mod D=128):
    x[b, j]  = (c1[j] * a[b, j+1] - c2[j] * a[b, j-2]) * a[b, j-1]
    out      = x + a @ W_lin.T + b_lin
with c1[j] = w[j,0]*outer[j], c2[j] = w[j,1]*outer[j], outer[j] = w[j,2]
except j==1 where outer is w[1,0].

Defining G[j, (j+1)%D] = c1[j], G[j, (j-2)%D] = -c2[j]:
    g = a @ G.T;  x[b, j] = g[b, j] * a[b, j-1];  out = x + a @ W_lin.T + b_lin

Strategy: pure data parallel across 8 NeuronCores (batch split), computing in
FEATURE-TRANSPOSED space with a row-rolled coordinate change so every on-chip
elementwise op is perfectly aligned:

  - The host uploads aT = shard.T as bf16 [128, nrows]: features live on
    partitions, batch on the free dim.  No on-chip transpose is needed, and
    the j-1 stencil shift becomes a partition shift, which a row-roll of the
    CONSTANT matrices absorbs:
        Gs = roll(G, -1, rows); Ws = roll(W_lin, -1, rows); bs = roll(b_lin, -1)
        gs  = Gs @ aT           (gs[j]  = g^T[j+1])
        ms  = Ws @ aT + bs      (ms[j]  = (a@W^T + b)^T[j+1])
        xs  = aT * gs           (aligned elementwise, no wrap)
        outs = xs + ms          (outs[j] = out^T[j+1])
    The host un-rolls the output: out = roll(outs, +1, rows).T.
  - Per 1024-col subtile: 4 bf16 matmuls (N=512 each, stationary Gs^T / Ws^T
    consts) write gs/ms to PSUM fp32; the evacuation to SBUF bf16 runs at 1x
    (fp32 PSUM source), so it is split between ScalarE and DVE; the bias add
    rides the ms evacuation (ScalarE per-partition activation bias).
  - The mul runs on DVE as bf16 tensor_tensor in 2x_1P mode (16-bit, step 1,
    4B-aligned) over 2048-col groups; the add is split DVE / GpSimd (GpSimd
    is SBUF-only but otherwise idle).
  - bf16 halves the DMA traffic (16 MiB in + 16 MiB out per core).  Consts
    load first on the SP HWDGE ring (so they are not starved behind data
    tiles), then 4 warm-up 2048-col tiles let compute start early, then
    8192-col tiles; stores ride the ACT ring in 4096-col tiles.
"""

import os
import sys

import numpy as np

if os.path.isdir("/opt/trn_rl_repo") and "/opt/trn_rl_repo" not in sys.path:
    sys.path.insert(0, "/opt/trn_rl_repo")

import ml_dtypes

import concourse.mybir as mybir
import concourse.tile as tile
from concourse import bacc
from concourse.bass_utils import run_bass_kernel_spmd

D = 128            # feature dim
N_CORES = 8
SUB = 1024         # cols per PSUM subtile (2 banks per tensor)
GRP = 2048         # cols per DVE group (2 subtiles)
T_WARM = 2048      # cols per warm-up load tile (x4)
N_WARM = 4
T_LOAD = 8192      # cols per steady-state load DMA tile
T_STORE = 4096     # cols per store DMA tile
K_SAFE = 8         # warm-up subtiles on the race-free (non-fused) path
BF16 = mybir.dt.bfloat16
F32 = mybir.dt.float32
NP_BF16 = ml_dtypes.bfloat16


def build_program(nrows: int):
    """Build the single-core Bass program (SPMD across cores)."""
    assert (nrows - N_WARM * T_WARM) % T_LOAD == 0 and nrows % GRP == 0
    ngrp = nrows // GRP

    nc = bacc.Bacc("TRN2", debug=False, target_bir_lowering=False)

    at = nc.declare_dram_parameter("at", [D, nrows], BF16, isOutput=False)
    gst = nc.declare_dram_parameter("gst", [D, D], BF16, isOutput=False)
    wst = nc.declare_dram_parameter("wst", [D, D], BF16, isOutput=False)
    bs = nc.declare_dram_parameter("bs", [D, 1], F32, isOutput=False)
    out = nc.declare_dram_parameter("out", [D, nrows], BF16, isOutput=True)

    with tile.TileContext(nc) as tc:
        with (
            tc.tile_pool(name="const", bufs=1) as const_pool,
            tc.tile_pool(name="aw_sb", bufs=N_WARM) as aw_pool,
            tc.tile_pool(name="a_sb", bufs=3) as a_pool,
            tc.tile_pool(name="o_sb", bufs=3) as o_pool,
            tc.tile_pool(name="xs_sb", bufs=3) as xs_pool,
            tc.tile_pool(name="gs_ps", bufs=2, space="PSUM") as gsps_pool,
            tc.tile_pool(name="ms_ps", bufs=2, space="PSUM") as msps_pool,
        ):
            # --- constants (tiny), FIRST on the SP ring: they complete
            # before the first data tile (FIFO) instead of trickling at
            # packet granularity behind 2 MiB loads ---
            gst_sb = const_pool.tile([D, D], BF16)
            wst_sb = const_pool.tile([D, D], BF16)
            wk0_sb = const_pool.tile([D, D], BF16)
            wk1_sb = const_pool.tile([D, D], BF16)
            wk2_sb = const_pool.tile([D, D], BF16)
            bs_sb = const_pool.tile([D, 1], F32)
            nc.sync.dma_start(out=gst_sb[:], in_=gst[:, :])
            nc.sync.dma_start(out=wst_sb[:], in_=wst[:, :])
            nc.sync.dma_start(out=wk0_sb[:], in_=wst[:, :])
            nc.sync.dma_start(out=wk1_sb[:], in_=wst[:, :])
            nc.sync.dma_start(out=wk2_sb[:], in_=wst[:, :])
            nc.sync.dma_start(out=bs_sb[:], in_=bs[:, :])
            wks = [wk0_sb, wk1_sb, wk2_sb]

            # Per-subtile software pipeline, staged so every engine's queue
            # stays hazard-free:
            #   A(k): PE   gs_ps(k) = Gs @ aT(k)          (2 matmuls)
            #   B(k): DVE  ms_ps(k) = aT(k) * gs_ps(k)    (xs, fp32 in PSUM)
            #   C(k): PE   ms_ps(k) += Ws @ aT(k)         (2 matmuls,
            #              start=False accumulate on top of xs)
            #   D(k): Sc   o(k) = bf16(ms_ps(k) + bs)     (activation bias)
            #   E:    store per T_STORE cols
            # PE's queue per step is [A(k), C(k-1)] so the PE never waits
            # long on B; PSUM: gs 2 bufs + ms 2 bufs = 8 banks exactly.
            nsub = nrows // SUB
            st = {}  # k -> (a_sb, o_sb, ga, go)
            a_sb, o_sb = None, None

            def emit_A_B(k):
                nonlocal a_sb, o_sb
                c0 = k * SUB
                # --- load (SP HWDGE ring): warm-up tiles then 8 KiB tiles ---
                if c0 < N_WARM * T_WARM:
                    if c0 % T_WARM == 0:
                        a_sb = aw_pool.tile([D, T_WARM], BF16, tag="aw")
                        nc.sync.dma_start(
                            out=a_sb[:], in_=at[:, c0 : c0 + T_WARM]
                        )
                    ga = c0 % T_WARM
                else:
                    if (c0 - N_WARM * T_WARM) % T_LOAD == 0:
                        a_sb = a_pool.tile([D, T_LOAD], BF16, tag="a")
                        nc.sync.dma_start(
                            out=a_sb[:], in_=at[:, c0 : c0 + T_LOAD]
                        )
                    ga = (c0 - N_WARM * T_WARM) % T_LOAD
                if c0 % T_STORE == 0:
                    o_sb = o_pool.tile([D, T_STORE], BF16, tag="o")
                go = c0 % T_STORE
                st[k] = (a_sb, o_sb, ga, go)

                gs_ps = gsps_pool.tile([D, SUB], F32, tag="gsps")
                ms_ps = msps_pool.tile([D, SUB], F32, tag="msps")
                for h in range(2):
                    hs = h * 512
                    nc.tensor.matmul(
                        out=gs_ps[:, hs : hs + 512],
                        lhsT=gst_sb[:],
                        rhs=a_sb[:, ga + hs : ga + hs + 512],
                        start=True,
                        stop=True,
                    )
                if k < K_SAFE:
                    # warm-up subtiles: race-free path -- xs to SBUF, the
                    # combine happens in D via a DVE STT (all deps are
                    # ordinary RAW; no PE accumulate onto DVE data)
                    xs_sb = xs_pool.tile([D, SUB], BF16, tag="xs")
                    nc.vector.tensor_mul(
                        out=xs_sb[:], in0=a_sb[:, ga : ga + SUB], in1=gs_ps[:]
                    )
                else:
                    xs_sb = None
                    # xs straight into the ms PSUM tile (fp32, 1x)
                    nc.vector.tensor_mul(
                        out=ms_ps[:], in0=a_sb[:, ga : ga + SUB], in1=gs_ps[:]
                    )
                st[k] += (gs_ps, ms_ps, xs_sb)

            def emit_C(k):
                a_sb, _, ga, _, _, ms_ps, xs_sb = st[k]
                safe = xs_sb is not None
                wk_sb = wks[k % 3]
                if not safe:
                    # The scheduler reorders engine queues freely subject to
                    # data deps, so the cross-engine WAW (DVE xs write in
                    # B(k) -> PE accumulate below) needs a REAL dataflow
                    # chain: rewrite column 0 of the ms matmuls' stationary
                    # tile as (xs[:,0:1] * 0.0) + wst[:,0:1] -- same values,
                    # but it READS the mul's output, so mul(k) -> stt(k) ->
                    # msmm(k) is enforced by ordinary RAW tracking.  Rotate
                    # across 3 wk tiles so the WAR against the previous
                    # subtiles' lhsT reads has slack (no latency chain).
                    nc.vector.scalar_tensor_tensor(
                        out=wk_sb[:, 0:1],
                        in0=ms_ps[:, 0:1],
                        scalar=0.0,
                        in1=wst_sb[:, 0:1],
                        op0=mybir.AluOpType.mult,
                        op1=mybir.AluOpType.add,
                    )
                for h in range(2):
                    hs = h * 512
                    nc.tensor.matmul(
                        out=ms_ps[:, hs : hs + 512],
                        lhsT=wst_sb[:] if safe else wk_sb[:],
                        rhs=a_sb[:, ga + hs : ga + hs + 512],
                        start=safe,
                        stop=True,
                        skip_group_check=not safe,
                    )

            def emit_D_E(k):
                _, o_sb, _, go, _, ms_ps, xs_sb = st[k]
                if xs_sb is not None:
                    nc.vector.scalar_tensor_tensor(
                        out=o_sb[:, go : go + SUB],
                        in0=xs_sb[:],
                        scalar=bs_sb[:],
                        in1=ms_ps[:],
                        op0=mybir.AluOpType.add,
                        op1=mybir.AluOpType.add,
                    )
                else:
                    nc.scalar.add(
                        out=o_sb[:, go : go + SUB], in_=ms_ps[:], add=bs_sb[:]
                    )
                c1 = (k + 1) * SUB
                if c1 % T_STORE == 0:
                    nc.scalar.dma_start(
                        out=out[:, c1 - T_STORE : c1], in_=o_sb[:]
                    )
                del st[k]

            for step in range(nsub + 2):
                if step < nsub:
                    emit_A_B(step)
                if 1 <= step <= nsub:
                    emit_C(step - 1)
                if step >= 2:
                    emit_D_E(step - 2)

    nc.compile()
    return nc


def make_consts(w: np.ndarray, W_lin: np.ndarray, b_lin: np.ndarray):
    """Host-side constant preparation (all tiny)."""
    w = np.asarray(w, np.float64)
    c1 = w[:, 0] * w[:, 2]
    c2 = w[:, 1] * w[:, 2]
    # column 1 uses w[1,0] as the outer factor (faithful to source)
    c1[1] = w[1, 0] * w[1, 0]
    c2[1] = w[1, 1] * w[1, 0]

    j = np.arange(D)
    G = np.zeros((D, D), np.float64)
    G[j, (j + 1) % D] += c1
    G[j, (j - 2) % D] -= c2

    Gs = np.roll(G, -1, axis=0)
    Ws = np.roll(np.asarray(W_lin, np.float64), -1, axis=0)
    bsv = np.roll(np.asarray(b_lin, np.float32), -1)

    gst = np.ascontiguousarray(Gs.T).astype(NP_BF16)
    wst = np.ascontiguousarray(Ws.T).astype(NP_BF16)
    bs = np.ascontiguousarray(bsv[:, None].astype(np.float32))
    return {"gst": gst, "wst": wst, "bs": bs}


_PROGRAM_CACHE: dict[int, object] = {}
TRACE = False      # test-only: capture NTFF profile on the next kernel() call
TRACE_DIR = None   # test-only: where to keep NTFF/perfetto artifacts
LAST_RESULT = None  # test-only: BassKernelResults of the last run


def _get_program(nrows: int):
    if nrows not in _PROGRAM_CACHE:
        _PROGRAM_CACHE[nrows] = build_program(nrows)
    return _PROGRAM_CACHE[nrows]


def kernel(**inputs) -> np.ndarray:
    inp = np.asarray(inputs["inp"])
    w = np.asarray(inputs["w"], np.float32)
    W_lin = np.asarray(inputs["W_lin"], np.float32)
    b_lin = np.asarray(inputs["b_lin"], np.float32)

    B = inp.shape[0]
    assert inp.shape[1] == D and B % N_CORES == 0
    nrows = B // N_CORES

    consts = make_consts(w, W_lin, b_lin)
    inp_bf = inp.astype(NP_BF16)

    nc = _get_program(nrows)
    in_maps = []
    for i in range(N_CORES):
        at = np.ascontiguousarray(inp_bf[i * nrows : (i + 1) * nrows, :].T)
        in_maps.append({"at": at, **consts})
    res = run_bass_kernel_spmd(
        nc, in_maps, list(range(N_CORES)), trace=TRACE, tmpdir=TRACE_DIR
    )
    global LAST_RESULT
    LAST_RESULT = res
    outs = [
        np.roll(np.asarray(res.results[i]["out"]), 1, axis=0).T.astype(np.float32)
        for i in range(N_CORES)
    ]
    return np.ascontiguousarray(np.concatenate(outs, axis=0))


if __name__ == "__main__":
    # quick smoke test on random data vs numpy
    rng = np.random.default_rng(0)
    B = N_CORES * (N_WARM * T_WARM + T_LOAD)
    inp = rng.standard_normal((B, D)).astype(np.float32)
    w = rng.random((D, 3)).astype(np.float32)
    W_lin = (rng.standard_normal((D, D)) / np.sqrt(D)).astype(np.float32)
    b_lin = (rng.standard_normal(D) * 0.01).astype(np.float32)
    dt = np.ones(1, np.float32)

    actual = kernel(inp=inp, dt=dt, w=w, W_lin=W_lin, b_lin=b_lin)

    a = inp.astype(np.float64)
    c1 = (w[:, 0] * w[:, 2]).astype(np.float64)
    c2 = (w[:, 1] * w[:, 2]).astype(np.float64)
    c1[1] = w[1, 0] * w[1, 0]
    c2[1] = w[1, 1] * w[1, 0]
    ap1 = np.roll(a, -1, 1)
    am2 = np.roll(a, 2, 1)
    am1 = np.roll(a, 1, 1)
    x = (c1 * ap1 - c2 * am2) * am1
    expected = x + a @ W_lin.astype(np.float64).T + b_lin
    err = np.abs(actual - expected).max() / np.abs(expected).max()
    print("scale-relative absmax err:", err)


# revision 20
# speedup vs baseline: 1.0397x; 1.0171x over previous
"""Trainium2 Bass kernel for the BINN convnet problem (transposed bf16 design).

Computation (per row b of inp, all column indices mod D=128):
    x[b, j]  = (c1[j] * a[b, j+1] - c2[j] * a[b, j-2]) * a[b, j-1]
    out      = x + a @ W_lin.T + b_lin
with c1[j] = w[j,0]*outer[j], c2[j] = w[j,1]*outer[j], outer[j] = w[j,2]
except j==1 where outer is w[1,0].

Defining G[j, (j+1)%D] = c1[j], G[j, (j-2)%D] = -c2[j]:
    g = a @ G.T;  x[b, j] = g[b, j] * a[b, j-1];  out = x + a @ W_lin.T + b_lin

Strategy: pure data parallel across 8 NeuronCores (batch split), computing in
FEATURE-TRANSPOSED space with a row-rolled coordinate change so every on-chip
elementwise op is perfectly aligned:

  - The host uploads aT = shard.T as bf16 [128, nrows]: features live on
    partitions, batch on the free dim.  No on-chip transpose is needed, and
    the j-1 stencil shift becomes a partition shift, which a row-roll of the
    CONSTANT matrices absorbs:
        Gs = roll(G, -1, rows); Ws = roll(W_lin, -1, rows); bs = roll(b_lin, -1)
        gs  = Gs @ aT           (gs[j]  = g^T[j+1])
        ms  = Ws @ aT + bs      (ms[j]  = (a@W^T + b)^T[j+1])
        xs  = aT * gs           (aligned elementwise, no wrap)
        outs = xs + ms          (outs[j] = out^T[j+1])
    The host un-rolls the output: out = roll(outs, +1, rows).T.
  - Per 1024-col subtile: 4 bf16 matmuls (N=512 each, stationary Gs^T / Ws^T
    consts) write gs/ms to PSUM fp32; the evacuation to SBUF bf16 runs at 1x
    (fp32 PSUM source), so it is split between ScalarE and DVE; the bias add
    rides the ms evacuation (ScalarE per-partition activation bias).
  - The mul runs on DVE as bf16 tensor_tensor in 2x_1P mode (16-bit, step 1,
    4B-aligned) over 2048-col groups; the add is split DVE / GpSimd (GpSimd
    is SBUF-only but otherwise idle).
  - bf16 halves the DMA traffic (16 MiB in + 16 MiB out per core).  Consts
    load first on the SP HWDGE ring (so they are not starved behind data
    tiles), then 4 warm-up 2048-col tiles let compute start early, then
    8192-col tiles; stores ride the ACT ring in 4096-col tiles.
"""

import os
import sys

import numpy as np

if os.path.isdir("/opt/trn_rl_repo") and "/opt/trn_rl_repo" not in sys.path:
    sys.path.insert(0, "/opt/trn_rl_repo")

import ml_dtypes

import concourse.mybir as mybir
import concourse.tile as tile
from concourse import bacc
from concourse.bass_utils import run_bass_kernel_spmd

D = 128            # feature dim
N_CORES = 8
SUB = 1024         # cols per PSUM subtile (2 banks per tensor)
GRP = 2048         # cols per DVE group (2 subtiles)
T_WARM = 2048      # cols per warm-up load tile (x4)
N_WARM = 4
T_LOAD = 8192      # cols per steady-state load DMA tile
T_STORE = 4096     # cols per store DMA tile
K_SAFE = 8         # warm-up subtiles on the race-free (non-fused) path
BF16 = mybir.dt.bfloat16
F32 = mybir.dt.float32
NP_BF16 = ml_dtypes.bfloat16


def build_program(nrows: int):
    """Build the single-core Bass program (SPMD across cores)."""
    assert (nrows - N_WARM * T_WARM) % T_LOAD == 0 and nrows % GRP == 0
    ngrp = nrows // GRP

    nc = bacc.Bacc("TRN2", debug=False, target_bir_lowering=False)

    at = nc.declare_dram_parameter("at", [D, nrows], BF16, isOutput=False)
    gst = nc.declare_dram_parameter("gst", [D, D], BF16, isOutput=False)
    wst = nc.declare_dram_parameter("wst", [D, D], BF16, isOutput=False)
    bs = nc.declare_dram_parameter("bs", [D, 1], F32, isOutput=False)
    out = nc.declare_dram_parameter("out", [D, nrows], BF16, isOutput=True)

    with tile.TileContext(nc) as tc:
        with (
            tc.tile_pool(name="const", bufs=1) as const_pool,
            tc.tile_pool(name="aw_sb", bufs=N_WARM) as aw_pool,
            tc.tile_pool(name="a_sb", bufs=3) as a_pool,
            tc.tile_pool(name="o_sb", bufs=3) as o_pool,
            tc.tile_pool(name="xs_sb", bufs=3) as xs_pool,
            tc.tile_pool(name="gs_ps", bufs=4, space="PSUM") as gsps_pool,
        ):
            # --- constants (tiny), FIRST on the SP ring: they complete
            # before the first data tile (FIFO) instead of trickling at
            # packet granularity behind 2 MiB loads ---
            gst_sb = const_pool.tile([D, D], BF16)
            wst_sb = const_pool.tile([D, D], BF16)
            wk0_sb = const_pool.tile([D, D], BF16)
            wk1_sb = const_pool.tile([D, D], BF16)
            wk2_sb = const_pool.tile([D, D], BF16)
            bs_sb = const_pool.tile([D, 1], F32)
            nc.sync.dma_start(out=gst_sb[:], in_=gst[:, :])
            nc.sync.dma_start(out=wst_sb[:], in_=wst[:, :])
            nc.sync.dma_start(out=wk0_sb[:], in_=wst[:, :])
            nc.sync.dma_start(out=wk1_sb[:], in_=wst[:, :])
            nc.sync.dma_start(out=wk2_sb[:], in_=wst[:, :])
            nc.sync.dma_start(out=bs_sb[:], in_=bs[:, :])
            wks = [wk0_sb, wk1_sb, wk2_sb]

            # Per-subtile software pipeline, staged so every engine's queue
            # stays hazard-free:
            #   A(k): PE   gs_ps(k) = Gs @ aT(k)          (2 matmuls)
            #   B(k): DVE  ms_ps(k) = aT(k) * gs_ps(k)    (xs, fp32 in PSUM)
            #   C(k): PE   ms_ps(k) += Ws @ aT(k)         (2 matmuls,
            #              start=False accumulate on top of xs)
            #   D(k): Sc   o(k) = bf16(ms_ps(k) + bs)     (activation bias)
            #   E:    store per T_STORE cols
            # PE's queue per step is [A(k), C(k-1)] so the PE never waits
            # long on B; PSUM: gs 2 bufs + ms 2 bufs = 8 banks exactly.
            nsub = nrows // SUB
            st = {}  # k -> (a_sb, o_sb, ga, go)
            a_sb, o_sb = None, None

            def emit_A_B(k):
                nonlocal a_sb, o_sb
                c0 = k * SUB
                # --- load (SP HWDGE ring): warm-up tiles then 8 KiB tiles ---
                if c0 < N_WARM * T_WARM:
                    if c0 % T_WARM == 0:
                        a_sb = aw_pool.tile([D, T_WARM], BF16, tag="aw")
                        nc.sync.dma_start(
                            out=a_sb[:], in_=at[:, c0 : c0 + T_WARM]
                        )
                    ga = c0 % T_WARM
                else:
                    if (c0 - N_WARM * T_WARM) % T_LOAD == 0:
                        a_sb = a_pool.tile([D, T_LOAD], BF16, tag="a")
                        nc.sync.dma_start(
                            out=a_sb[:], in_=at[:, c0 : c0 + T_LOAD]
                        )
                    ga = (c0 - N_WARM * T_WARM) % T_LOAD
                if c0 % T_STORE == 0:
                    o_sb = o_pool.tile([D, T_STORE], BF16, tag="o")
                go = c0 % T_STORE
                st[k] = (a_sb, o_sb, ga, go)

                gs_ps = gsps_pool.tile([D, SUB], F32, tag="gsps")
                for h in range(2):
                    hs = h * 512
                    nc.tensor.matmul(
                        out=gs_ps[:, hs : hs + 512],
                        lhsT=gst_sb[:],
                        rhs=a_sb[:, ga + hs : ga + hs + 512],
                        start=True,
                        stop=True,
                    )
                if k < K_SAFE:
                    # warm-up subtiles: race-free path -- xs to SBUF, ms in
                    # its own PSUM tile, the combine happens in D via a DVE
                    # STT (all deps are ordinary RAW; no PE accumulate onto
                    # DVE data)
                    ms_ps = gsps_pool.tile([D, SUB], F32, tag="gsps")
                    xs_sb = xs_pool.tile([D, SUB], BF16, tag="xs")
                    nc.vector.tensor_mul(
                        out=xs_sb[:], in0=a_sb[:, ga : ga + SUB], in1=gs_ps[:]
                    )
                else:
                    ms_ps = gs_ps
                    xs_sb = None
                    # xs in-place into the same PSUM tile (DVE RMW; the
                    # 8-stage pipe makes read-then-write of each element
                    # safe), the ms matmuls then accumulate on top
                    nc.vector.tensor_mul(
                        out=gs_ps[:], in0=a_sb[:, ga : ga + SUB], in1=gs_ps[:]
                    )
                st[k] += (gs_ps, ms_ps, xs_sb)

            def emit_C(k):
                a_sb, _, ga, _, _, ms_ps, xs_sb = st[k]
                safe = xs_sb is not None
                wk_sb = wks[k % 3]
                if not safe:
                    # The scheduler reorders engine queues freely subject to
                    # data deps, so the cross-engine WAW (DVE xs write in
                    # B(k) -> PE accumulate below) needs a REAL dataflow
                    # chain: rewrite column 0 of the ms matmuls' stationary
                    # tile as (xs[:,0:1] * 0.0) + wst[:,0:1] -- same values,
                    # but it READS the mul's output, so mul(k) -> stt(k) ->
                    # msmm(k) is enforced by ordinary RAW tracking.  Rotate
                    # across 3 wk tiles so the WAR against the previous
                    # subtiles' lhsT reads has slack (no latency chain).
                    nc.vector.scalar_tensor_tensor(
                        out=wk_sb[:, 0:1],
                        in0=ms_ps[:, 0:1],
                        scalar=0.0,
                        in1=wst_sb[:, 0:1],
                        op0=mybir.AluOpType.mult,
                        op1=mybir.AluOpType.add,
                    )
                for h in range(2):
                    hs = h * 512
                    nc.tensor.matmul(
                        out=ms_ps[:, hs : hs + 512],
                        lhsT=wst_sb[:] if safe else wk_sb[:],
                        rhs=a_sb[:, ga + hs : ga + hs + 512],
                        start=safe,
                        stop=True,
                        skip_group_check=not safe,
                    )

            def emit_D_E(k):
                _, o_sb, _, go, _, ms_ps, xs_sb = st[k]
                if xs_sb is not None:
                    nc.vector.scalar_tensor_tensor(
                        out=o_sb[:, go : go + SUB],
                        in0=xs_sb[:],
                        scalar=bs_sb[:],
                        in1=ms_ps[:],
                        op0=mybir.AluOpType.add,
                        op1=mybir.AluOpType.add,
                    )
                else:
                    nc.scalar.add(
                        out=o_sb[:, go : go + SUB], in_=ms_ps[:], add=bs_sb[:]
                    )
                c1 = (k + 1) * SUB
                if c1 % T_STORE == 0:
                    nc.scalar.dma_start(
                        out=out[:, c1 - T_STORE : c1], in_=o_sb[:]
                    )
                del st[k]

            for step in range(nsub + 2):
                if step < nsub:
                    emit_A_B(step)
                if 1 <= step <= nsub:
                    emit_C(step - 1)
                if step >= 2:
                    emit_D_E(step - 2)

    nc.compile()
    return nc


def make_consts(w: np.ndarray, W_lin: np.ndarray, b_lin: np.ndarray):
    """Host-side constant preparation (all tiny)."""
    w = np.asarray(w, np.float64)
    c1 = w[:, 0] * w[:, 2]
    c2 = w[:, 1] * w[:, 2]
    # column 1 uses w[1,0] as the outer factor (faithful to source)
    c1[1] = w[1, 0] * w[1, 0]
    c2[1] = w[1, 1] * w[1, 0]

    j = np.arange(D)
    G = np.zeros((D, D), np.float64)
    G[j, (j + 1) % D] += c1
    G[j, (j - 2) % D] -= c2

    Gs = np.roll(G, -1, axis=0)
    Ws = np.roll(np.asarray(W_lin, np.float64), -1, axis=0)
    bsv = np.roll(np.asarray(b_lin, np.float32), -1)

    gst = np.ascontiguousarray(Gs.T).astype(NP_BF16)
    wst = np.ascontiguousarray(Ws.T).astype(NP_BF16)
    bs = np.ascontiguousarray(bsv[:, None].astype(np.float32))
    return {"gst": gst, "wst": wst, "bs": bs}


_PROGRAM_CACHE: dict[int, object] = {}
TRACE = False      # test-only: capture NTFF profile on the next kernel() call
TRACE_DIR = None   # test-only: where to keep NTFF/perfetto artifacts
LAST_RESULT = None  # test-only: BassKernelResults of the last run


def _get_program(nrows: int):
    if nrows not in _PROGRAM_CACHE:
        _PROGRAM_CACHE[nrows] = build_program(nrows)
    return _PROGRAM_CACHE[nrows]


def kernel(**inputs) -> np.ndarray:
    inp = np.asarray(inputs["inp"])
    w = np.asarray(inputs["w"], np.float32)
    W_lin = np.asarray(inputs["W_lin"], np.float32)
    b_lin = np.asarray(inputs["b_lin"], np.float32)

    B = inp.shape[0]
    assert inp.shape[1] == D and B % N_CORES == 0
    nrows = B // N_CORES

    consts = make_consts(w, W_lin, b_lin)
    inp_bf = inp.astype(NP_BF16)

    nc = _get_program(nrows)
    in_maps = []
    for i in range(N_CORES):
        at = np.ascontiguousarray(inp_bf[i * nrows : (i + 1) * nrows, :].T)
        in_maps.append({"at": at, **consts})
    res = run_bass_kernel_spmd(
        nc, in_maps, list(range(N_CORES)), trace=TRACE, tmpdir=TRACE_DIR
    )
    global LAST_RESULT
    LAST_RESULT = res
    outs = [
        np.roll(np.asarray(res.results[i]["out"]), 1, axis=0).T.astype(np.float32)
        for i in range(N_CORES)
    ]
    return np.ascontiguousarray(np.concatenate(outs, axis=0))


if __name__ == "__main__":
    # quick smoke test on random data vs numpy
    rng = np.random.default_rng(0)
    B = N_CORES * (N_WARM * T_WARM + T_LOAD)
    inp = rng.standard_normal((B, D)).astype(np.float32)
    w = rng.random((D, 3)).astype(np.float32)
    W_lin = (rng.standard_normal((D, D)) / np.sqrt(D)).astype(np.float32)
    b_lin = (rng.standard_normal(D) * 0.01).astype(np.float32)
    dt = np.ones(1, np.float32)

    actual = kernel(inp=inp, dt=dt, w=w, W_lin=W_lin, b_lin=b_lin)

    a = inp.astype(np.float64)
    c1 = (w[:, 0] * w[:, 2]).astype(np.float64)
    c2 = (w[:, 1] * w[:, 2]).astype(np.float64)
    c1[1] = w[1, 0] * w[1, 0]
    c2[1] = w[1, 1] * w[1, 0]
    ap1 = np.roll(a, -1, 1)
    am2 = np.roll(a, 2, 1)
    am1 = np.roll(a, 1, 1)
    x = (c1 * ap1 - c2 * am2) * am1
    expected = x + a @ W_lin.astype(np.float64).T + b_lin
    err = np.abs(actual - expected).max() / np.abs(expected).max()
    print("scale-relative absmax err:", err)
